# revision 1
# baseline (speedup 1.0000x reference)
# Trainium2 Bass kernel for nn_Encoder_SelfAttention (sparse_attention).
#
# Contract: kernel(**inputs) takes the FULL unsharded inputs (see shapes
# below) and returns the FULL (8, 512, 512) float32 output. Internally the
# batch dimension (8) is sharded one-batch-per-core across 8 NeuronCores;
# parameters are replicated. The whole forward pass runs on-device; the host
# only prepares layout/constant matrices (FFT bases, log-distance matrices,
# triangular sign masks, identity) and gathers per-core outputs.
#
# Math notes (validated against the reference to ~1e-6 rel):
# - order bias: log(pr)*gd + log(1-pr)*(1-gd) == -softplus(s*x) with
#   x = oq[q]+ok[k]+b and s = -1 where k>q (T layout) else +1, because
#   log(sigmoid(x)) = -softplus(-x). softplus(z) = ln(1+exp(z)) via the
#   exp/ln ACT table set.
# - dist bias: -c*(g-p)^2 with p = dk'[k]+dq[q] expands into PE-friendly
#   terms: -c*g^2 (identity-scaled matmul), 2c*diag(dk')@g, g@diag(2c*dq)
#   (g is symmetric), and a rank-3 outer product for -c*p^2. Everything
#   accumulates in PSUM for free.
# - scores are built TRANSPOSED ([k, q] with k on partitions) so softmax
#   renormalization sums via a ones-column appended to V in the ctx matmul
#   and every matmul operand stays in its natural layout (no probs
#   transpose).
# - rfft/filter/irfft over the hidden dim is 4 real matmuls against
#   host-built ortho DFT bases; the inverse also accumulates the +ctx
#   residual via identity-matmul transpose into the same PSUM tile.
# - 1/sqrt(var+eps) and 1/denom are computed as exp(-0.5*ln(.)) and
#   exp(-ln(.)) to stay inside the single natural_log_exp ACT table set.
import sys

sys.path.insert(0, "/opt/trn_rl_repo")

import numpy as np
from contextlib import ExitStack

import concourse.bass as bass
import concourse.tile as tile
from concourse import mybir
from concourse.bass_utils import run_bass_kernel_spmd
from concourse.masks import make_identity
from concourse.vector_clock import ScopedClock, VectorClock

F32 = mybir.dt.float32
AF = mybir.ActivationFunctionType
B, S, H, NH, D = 8, 512, 512, 8, 64
NT = 4  # 128-partition tiles per 512
FT_SIZES = (128, 128, 1)  # rfft freq tiling (257 = 128+128+1)


class _TileContext(tile.TileContext):
    # This walrus build rejects sem waits attached to SP CTRL instructions
    # (Drain/NoOp) when more than one is present ("Too many sync wait
    # commands"). Split the tail-drain global-clock waits one-per-NOP.
    def _drain_and_barrier(self, tick_clock, wait_clock):
        g = tick_clock.global_clock
        n = len(g)
        for i in range(n):
            if g[i] > 0:
                vec = [0] * n
                vec[i] = g[i]
                nop_inst = self.nc.sync.nop(nofuse=True)
                wait_clock.add_sem_waits(
                    nop_inst.ins, ScopedClock({None: VectorClock(vec)})
                )
        self.nc.sync.drain()
        self.nc.all_engine_barrier()
        assert self.sems is not None
        popped = self.nc._tile_sem_poison_stack.pop()
        assert popped is self._sem_poison
        self.nc.clear_and_free_semaphores(list(self.sems.allocated().values()))
        self.nc.all_engine_barrier()


def _split_excess_waits(nc):
    """This walrus build allows at most 1 sync-wait per instruction (2 for
    EventSemaphore). Spill extras onto injected same-engine NOPs placed
    immediately before the over-subscribed instruction."""
    import bass_rust

    total = 0
    for fn in nc.m.functions:
        for blk in fn.blocks:
            out = []
            for inst in blk.instructions:
                si = inst.sync_info
                waits = list(si.on_wait) if si is not None else []
                cap = 2 if inst.__class__.__name__ == "InstEventSemaphore" else 1
                if len(waits) > cap:
                    keep, spill = waits[:cap], waits[cap:]
                    for w in spill:
                        nop = mybir.InstNoOp(
                            name=f"wsplit-{inst.name}-{total}", ins=[], outs=[])
                        nop.engine = inst.engine
                        nop.sync_info = bass_rust.SyncInfo(on_wait=[w], on_update=[])
                        out.append(nop)
                        total += 1
                    inst.sync_info = bass_rust.SyncInfo(
                        on_wait=keep, on_update=list(si.on_update))
                out.append(inst)
            blk.instructions = out
    return total


def _host_constants():
    """Input-independent structural constants."""
    W = np.fft.rfft(np.eye(H, dtype=np.float64), norm="ortho", axis=-1)
    cret = np.ascontiguousarray(W.real).astype(np.float32)  # [H, 257]
    cimt = np.ascontiguousarray(W.imag).astype(np.float32)
    irA = np.fft.irfft(np.eye(257, dtype=np.complex128), n=H, norm="ortho", axis=-1).astype(np.float32)
    irB = np.fft.irfft(1j * np.eye(257, dtype=np.complex128), n=H, norm="ortho", axis=-1).astype(np.float32)
    idx = np.arange(S)
    g = np.log(np.abs(idx[None, :] - idx[:, None]).astype(np.float64) + 1.0).astype(np.float32)
    g2 = (g.astype(np.float64) ** 2).astype(np.float32)
    ssign = np.where(idx[:, None] > idx[None, :], -1.0, 1.0).astype(np.float32)
    onesel = np.zeros((NH, NH * 128), np.float32)  # slice h: row h all-ones
    for h in range(NH):
        onesel[h, h * 128:(h + 1) * 128] = 1.0
    halfsel = np.zeros((1, 768), np.float32)  # [0:128]: first 64; [128:256]: last 64; [256:768]: ones row
    halfsel[0, 0:64] = 1.0
    halfsel[0, 192:256] = 1.0
    halfsel[0, 256:768] = 1.0
    return dict(cret=cret, cimt=cimt, irA=irA, irB=irB, g=g, g2=g2, ss=ssign,
                onesel=onesel, halfsel=halfsel)


def _build_program(c, flags):
    """Build the per-core Bass program. `c` holds baked scalar constants."""
    nc = bass.Bass("TRN2", target_bir_lowering=False, debug=False)
    dt = F32

    RDT = mybir.dt.float32r

    def din(name, shape=None, rdt=False):
        return nc.dram_tensor(name, list(shape), RDT if rdt else dt,
                              kind="ExternalInput").ap()

    x_d = din("x", (S, H))
    wq_d = din("wq", rdt=True, shape=(H, H))
    wk_d = din("wk", rdt=True, shape=(H, H))
    wv_d = din("wv", rdt=True, shape=(H, H))
    wblk_d = din("wblk", rdt=True, shape=(H, 32))
    gh_d = nc.dram_tensor("gh", [S, S], mybir.dt.float16, kind="ExternalInput").ap()
    g2_d = din("g2", rdt=True, shape=(S, S))
    ss_d = din("ss", (S, S))
    cret_d = din("cret", rdt=True, shape=(H, 257))
    cimt_d = din("cimt", rdt=True, shape=(H, 257))
    irA_d = din("irA", rdt=True, shape=(257, H))
    irB_d = din("irB", rdt=True, shape=(257, H))
    wrt_d = din("wrt", (257, S))
    wit_d = din("wit", (257, S))
    onesel_d = din("onesel", rdt=True, shape=(NH, NH * 128))
    halfsel_d = din("halfsel", rdt=True, shape=(1, 768))
    if flags["use_mask"]:
        m8_d = din("m8", (S,))
    if flags["use_bq"]:
        bq_d = din("bq", (H,))
    if flags["use_bk"]:
        bk_d = din("bk", (H,))
    if flags["use_bv"]:
        bv_d = din("bv", (H,))
    ln_bcast = {}
    for nm in ("lnfw", "lnfb", "lnw", "lnb"):
        if flags["use_" + nm]:
            ln_bcast[nm] = din(nm, (H,))
    out_d = nc.dram_tensor("out", [S, H], dt, kind="ExternalOutput").ap()

    negc = -c["c"]
    twoc = 2.0 * c["c"]
    fbase = (0, 128, 256)

    with _TileContext(nc) as tc:
        with ExitStack() as ctx:
            consts = ctx.enter_context(tc.tile_pool(name="consts", bufs=1))
            work = ctx.enter_context(tc.tile_pool(name="work", bufs=2))
            workw = ctx.enter_context(tc.tile_pool(name="workw", bufs=3))
            small = ctx.enter_context(tc.tile_pool(name="small", bufs=2))
            smallb = ctx.enter_context(tc.tile_pool(name="smallb", bufs=1))
            diagp = ctx.enter_context(tc.tile_pool(name="diagp", bufs=4))
            p1p = ctx.enter_context(tc.tile_pool(name="p1p", bufs=2, space="PSUM"))
            miscp = ctx.enter_context(tc.tile_pool(name="miscp", bufs=2, space="PSUM"))
            ctxp = ctx.enter_context(tc.tile_pool(name="ctxp", bufs=2, space="PSUM"))

            def load(dram_ap, shape, tag, engine=None, pool=None):
                t = (pool or consts).tile(list(shape), dram_ap.dtype, tag=tag, name=tag)
                if engine is None:
                    # phase-D constants ride the Activation HWDGE ring (idle
                    # early); everything else the SP ring
                    engine = nc.scalar if pool is not None else nc.sync
                engine.dma_start(t[:], dram_ap)
                return t

            R = mybir.dt.float32r

            def mmr(out, lhsT, rhs, **kw):
                # fp32r: same 4-byte data, 4x PE throughput vs fp32
                nc.tensor.matmul(out, lhsT.bitcast(R), rhs.bitcast(R), **kw)

            # ---- constant loads ----
            _ccols = {}

            def constcol(val):
                # [128,1] memset column for activation-bias immediates
                if val not in _ccols:
                    t = consts.tile([128, 1], dt, tag=f"cc{len(_ccols)}")
                    nc.vector.memset(t[:], val)
                    _ccols[val] = t
                return _ccols[val]

            i128 = consts.tile([128, 128], dt, tag="i128")
            make_identity(nc, i128[:])
            i128r = consts.tile([128, 128], dt, tag="i128r")
            nc.vector.tensor_copy(i128r[:].bitcast(mybir.dt.float32r), i128[:])
            negci = consts.tile([128, 128], dt, tag="negci")
            nc.vector.tensor_scalar_mul(negci[:].bitcast(R), i128[:], negc)

            x_t = [load(x_d[k * 128:(k + 1) * 128, :], (128, H), f"x{k}") for k in range(NT)]
            wblk_t = [load(wblk_d[k * 128:(k + 1) * 128, :], (128, 32), f"wblk{k}") for k in range(NT)]
            onesel_t = load(onesel_d[:], (NH, NH * 128), "onesel")
            halfsel_t = load(halfsel_d[:], (1, 768), "halfsel")
            ones_f = consts.tile([128, NH], dt, tag="ones_f")
            nc.vector.memset(ones_f[:], 1.0)
            ones_row = halfsel_t[0:1, 256:768]
            if flags["use_mask"]:
                m8c = consts.tile([128, NT], dt, tag="m8c")
                nc.sync.dma_start(m8c[:], bass.AP(tensor=m8_d.tensor, offset=0, ap=[[1, 128], [128, NT]]))
            bias_cols = {}
            if flags["use_bq"]:
                t = consts.tile([128, NT], dt, tag="bq")
                nc.sync.dma_start(t[:], bass.AP(tensor=bq_d.tensor, offset=0, ap=[[1, 128], [128, NT]]))
                bias_cols["bq"] = t
            if flags["use_bk"]:
                t = consts.tile([128, NT], dt, tag="bk")
                nc.sync.dma_start(t[:], bass.AP(tensor=bk_d.tensor, offset=0, ap=[[1, 128], [128, NT]]))
                bias_cols["bk"] = t
            if flags["use_bv"]:
                bv_row = load(bass.AP(tensor=bv_d.tensor, offset=0, ap=[[0, 1], [1, H]]), (1, H), "bv")
            ln_bc = {}
            for nm, d_ap in ln_bcast.items():
                t = consts.tile([128, H], dt, tag=nm + "b")
                nc.gpsimd.dma_start(t[:], bass.AP(tensor=d_ap.tensor, offset=0, ap=[[0, 128], [1, H]]))
                ln_bc[nm] = t

            # ---- phase A: X^T, projections (weights live in a scoped pool) ----
            qt_sb, kt_sb, vaug_sb = [], [], []
            with ExitStack() as wctx:
                wpool = wctx.enter_context(tc.tile_pool(name="wpool", bufs=1))
                wq_t = [load(wq_d[k * 128:(k + 1) * 128, :], (128, H), f"wq{k}", pool=wpool) for k in range(NT)]
                wk_t = [load(wk_d[k * 128:(k + 1) * 128, :], (128, H), f"wk{k}", pool=wpool) for k in range(NT)]
                wv_t = [load(wv_d[k * 128:(k + 1) * 128, :], (128, H), f"wv{k}", pool=wpool) for k in range(NT)]
                gh_t = [load(gh_d[k * 128:(k + 1) * 128, :], (128, S), f"gh{k}") for k in range(NT)]
                g2_t = [load(g2_d[k * 128:(k + 1) * 128, :], (128, S), f"g2{k}") for k in range(NT)]
                ss_t = [load(ss_d[k * 128:(k + 1) * 128, :], (128, S), f"ss{k}") for k in range(NT)]
                xt_sb = []
                for ht in range(NT):
                    ps = miscp.tile([128, S], dt, tag="m")
                    for st in range(NT):
                        nc.tensor.transpose(ps[:, st * 128:(st + 1) * 128],
                                            x_t[st][:, ht * 128:(ht + 1) * 128], i128[:])
                    t = wpool.tile([128, S], dt, tag=f"xt{ht}", name=f"xt{ht}")
                    nc.vector.tensor_copy(t[:].bitcast(R), ps[:])
                    xt_sb.append(t)

                def project_T(w_t, bias_col, tagp):
                    outs = []
                    for ot in range(NT):
                        ps = miscp.tile([128, S], dt, tag="m")
                        for ht in range(NT):
                            mmr(ps[:], w_t[ht][:, ot * 128:(ot + 1) * 128], xt_sb[ht][:],
                                             start=(ht == 0), stop=(ht == NT - 1))
                        t = consts.tile([128, S], dt, tag=f"{tagp}{ot}", name=f"{tagp}{ot}")
                        if bias_col is not None:
                            nc.scalar.activation(t[:].bitcast(R), ps[:], AF.Identity, bias=bias_col[:, ot:ot + 1], scale=1.0)
                        else:
                            nc.vector.tensor_copy(t[:].bitcast(R), ps[:])
                        outs.append(t)
                    return outs

                qt_sb = project_T(wq_t, bias_cols.get("bq"), "qt")
                kt_sb = project_T(wk_t, bias_cols.get("bk"), "kt")

                for st in range(NT):
                    ps = miscp.tile([128, S], dt, tag="m")
                    for ht in range(NT):
                        mmr(ps[:], xt_sb[ht][:, st * 128:(st + 1) * 128], wv_t[ht][:],
                                         start=(ht == 0), stop=(ht == NT - 1 and not flags["use_bv"]))
                    if flags["use_bv"]:
                        mmr(ps[:], halfsel_t[0:1, 256 + st * 128:256 + (st + 1) * 128], bv_row[:],
                                         start=False, stop=True)
                    t = consts.tile([128, NH * 65], dt, tag=f"vaug{st}", name=f"vaug{st}")
                    tap = t[:]
                    ones_cols = bass.AP(tensor=tap.tensor, offset=tap.offset + D,
                                        ap=[list(tap.ap[0]), [65, NH], [1, 1]])
                    nc.vector.tensor_copy(ones_cols.bitcast(R), ones_f[:])
                    dst = bass.AP(tensor=tap.tensor, offset=tap.offset,
                                  ap=[list(tap.ap[0]), [65, NH], [1, D]])
                    nc.vector.tensor_copy(dst.bitcast(R), ps[:])
                    vaug_sb.append(t)

            # ---- per-head rows oq', ok, dq, dk' ([8, S] each) ----
            row_specs = (("oq", qt_sb, 0, c["b_order"]), ("ok", kt_sb, 1, 0.0),
                         ("dq", qt_sb, 2, 0.0), ("dk", kt_sb, 3, c["b_dist"]))
            rows = {}
            for nm, src, ti, bias in row_specs:
                ps = miscp.tile([8, S], dt, tag="m")
                for ht in range(NT):
                    mmr(ps[:], wblk_t[ht][:, ti * 8:(ti + 1) * 8], src[ht][:],
                                     start=(ht == 0), stop=(ht == NT - 1))
                t = consts.tile([8, S], dt, tag=f"row{nm}")
                if bias != 0.0:
                    nc.scalar.activation(t[:].bitcast(R), ps[:], AF.Identity,
                                         bias=constcol(float(bias))[0:8, 0:1], scale=1.0)
                else:
                    nc.scalar.copy(t[:].bitcast(R), ps[:])
                rows[nm] = t

            n2cdq = consts.tile([8, S], dt, tag="n2cdq")  # -2c * dq
            nc.vector.tensor_scalar_mul(n2cdq[:].bitcast(R), rows["dq"][:], 2.0 * negc)

            # transposed per-position columns [128, 32] per kt: ok/dq/dk cols
            cols_sb = []
            for st in range(NT):
                ps = miscp.tile([128, 32], dt, tag="m")
                for ti, nm in enumerate(("oq", "ok", "dq", "dk")):
                    mmr(ps[:, ti * 8:(ti + 1) * 8],
                        rows[nm][:, st * 128:(st + 1) * 128], i128r[0:8, 0:8],
                        start=True, stop=True)
                t = consts.tile([128, 32], dt, tag=f"cols{st}")
                nc.scalar.copy(t[:], ps[:])
                cols_sb.append(t)
            c2dk = []
            c2dq = []
            nck = []  # S-op per-partition column: -c*dk'^2 (+ 8*mask)
            for st in range(NT):
                t = consts.tile([128, NH], dt, tag=f"c2dk{st}")
                nc.vector.tensor_scalar_mul(t[:], cols_sb[st][:, 24:32], twoc)
                c2dk.append(t)
                t = consts.tile([128, NH], dt, tag=f"c2dq{st}")
                nc.vector.tensor_scalar_mul(t[:], cols_sb[st][:, 16:24], twoc)
                c2dq.append(t)
                t = consts.tile([128, NH], dt, tag=f"nck{st}")
                nc.scalar.square(t[:], cols_sb[st][:, 24:32])
                nc.vector.tensor_scalar_mul(t[:], t[:], negc)
                if flags["use_mask"]:
                    nc.vector.tensor_scalar(t[:], t[:], m8c[:, st:st + 1], None,
                                            mybir.AluOpType.add)
                nck.append(t)

            # ---- phase B: per-head transposed scores + softmax + ctx ----
            ctxt_sb = [consts.tile([128, S], dt, tag=f"ctxt{ht}", name=f"ctxt{ht}") for ht in range(NT)]
            ctx_live = {}
            recips = {}
            for h in range(NH):
                ot, po = h // 2, (h % 2) * D
                ddq = []
                for qt in range(NT):
                    dq_tile = diagp.tile([128, 128], mybir.dt.float16, tag="ddq")
                    nc.gpsimd.tensor_scalar_mul(dq_tile[:], i128[:], c2dq[qt][:, h:h + 1])
                    ddq.append(dq_tile)
                p1a = p1p.tile([128, 1024], dt, tag="p1")
                p1b = p1p.tile([128, 1024], dt, tag="p1")
                w_h = workw.tile([128, NT * S], dt, tag="wh")
                # oq'[q] broadcast to all partitions: ones-selector matmul
                oqb_h = miscp.tile([128, S], dt, tag="m")
                mmr(oqb_h[:], onesel_t[:, h * 128:(h + 1) * 128], rows["oq"][:],
                                 start=True, stop=True)
                # delta-masked -2c*dq rows for the rank-1 cross matmul
                nmask_h = small.tile([NH, S], dt, tag="nmask")
                nc.vector.tensor_scalar_mul(nmask_h[:].bitcast(R), n2cdq[:], i128[0:8, h:h + 1])
                for kt in range(NT):
                    ptile = p1a if kt < 2 else p1b
                    o = ptile[:, (kt % 2) * S:(kt % 2) * S + S]
                    ksl = slice(kt * 128, (kt + 1) * 128)
                    mmr(o, kt_sb[ot][po:po + D, ksl], qt_sb[ot][po:po + D, :],
                                     start=True, stop=False)
                    mmr(o, negci[:], g2_t[kt][:], start=False, stop=False)
                    ddk = diagp.tile([128, 128], mybir.dt.float16, tag="ddk")
                    nc.gpsimd.tensor_scalar_mul(ddk[:], i128[:], c2dk[kt][:, h:h + 1])
                    nc.tensor.matmul(o, ddk[:], gh_t[kt][:], start=False, stop=False)
                    mmr(o, rows["dk"][:, ksl], nmask_h[:], start=False, stop=False)
                    for qt in range(NT):
                        nc.tensor.matmul(o[:, qt * 128:(qt + 1) * 128], gh_t[qt][:, ksl], ddq[qt][:],
                                         start=False, stop=(qt == NT - 1))
                    # z = sign * (oq'[q] + ok[k]);  fused add+mul on DVE
                    nc.vector.scalar_tensor_tensor(
                        w_h[:, kt * S:(kt + 1) * S], oqb_h[:], cols_sb[kt][:, 8 + h:9 + h],
                        ss_t[kt][:], op0=mybir.AluOpType.add, op1=mybir.AluOpType.mult)
                # softplus chain (in place, per kt-pair so the P1 slot
                # frees as early as possible): w = ln(1 + exp(w))
                for half, ptile in ((0, p1a), (1, p1b)):
                    hsl = slice(half * 2 * S, (half * 2 + 2) * S)
                    nc.scalar.activation(w_h[:, hsl], w_h[:, hsl], AF.Exp)
                    nc.scalar.activation(w_h[:, hsl], w_h[:, hsl], AF.Ln, bias=1.0, scale=1.0)
                    for kt in (half * 2, half * 2 + 1):
                        sl = slice(kt * S, (kt + 1) * S)
                        nc.vector.scalar_tensor_tensor(
                            w_h[:, sl], ptile[:, (kt % 2) * S:(kt % 2) * S + S],
                            nck[kt][:, h:h + 1], w_h[:, sl],
                            op0=mybir.AluOpType.add, op1=mybir.AluOpType.subtract)
                et_h = work.tile([128, NT * S], mybir.dt.float32r, tag="eth")
                nc.scalar.activation(et_h[:, 0:2 * S], w_h[:, 0:2 * S], AF.Exp, scale=0.125)
                nc.scalar.activation(et_h[:, 2 * S:], w_h[:, 2 * S:], AF.Exp, scale=0.125)
                # ctx^T (+denominator row) for this head
                cps = ctxp.tile([65, S], dt, tag="ctx")
                for kt in range(NT):
                    mmr(cps[:], vaug_sb[kt][:, h * 65:(h + 1) * 65],
                                     et_h[:, kt * S:(kt + 1) * S],
                                     start=(kt == 0), stop=(kt == NT - 1))
                # 1/denominator via exp(-ln(.)) straight from the PSUM row
                rcl = small.tile([1, S], dt, tag="recipl")
                nc.scalar.activation(rcl[:], cps[64:65, :], AF.Ln)
                rc = small.tile([1, S], mybir.dt.float32r, tag="recip")
                nc.scalar.activation(rc[:], rcl[:], AF.Exp, scale=-1.0)
                ctx_live[h] = cps
                recips[h] = rc
                if h % 2 == 1:
                    ht2 = h // 2
                    rbp = miscp.tile([128, S], dt, tag="m")
                    mmr(rbp[:], halfsel_t[0:1, 0:128], recips[h - 1][:],
                        start=True, stop=False)
                    mmr(rbp[:], halfsel_t[0:1, 128:256], recips[h][:],
                        start=False, stop=True)
                    rbs = smallb.tile([128, S], dt, tag="rbs")
                    nc.scalar.copy(rbs[:], rbp[:])
                    for hh in (h - 1, h):
                        nc.vector.tensor_mul(ctxt_sb[ht2][(hh % 2) * D:(hh % 2) * D + D, :].bitcast(R),
                                             ctx_live[hh][0:D, :], rbs[(hh % 2) * D:(hh % 2) * D + D, :])
                    ctx_live.clear()
                    recips.clear()

            # ---- phase D constants: pool created after wpool exit so the
            # allocator reuses the freed weight space ----
            fftp = ctx.enter_context(tc.tile_pool(name="fftp", bufs=1))
            cret_t = [load(cret_d[k * 128:(k + 1) * 128, :], (128, 257), f"cret{k}", pool=fftp) for k in range(NT)]
            cimt_t = [load(cimt_d[k * 128:(k + 1) * 128, :], (128, 257), f"cimt{k}", pool=fftp) for k in range(NT)]
            irA_t = [load(irA_d[fbase[f]:fbase[f] + FT_SIZES[f], :], (FT_SIZES[f], H), f"irA{f}", pool=fftp) for f in range(3)]
            irB_t = [load(irB_d[fbase[f]:fbase[f] + FT_SIZES[f], :], (FT_SIZES[f], H), f"irB{f}", pool=fftp) for f in range(3)]
            wrt_t = [load(wrt_d[fbase[f]:fbase[f] + FT_SIZES[f], :], (FT_SIZES[f], S), f"wrt{f}", pool=fftp) for f in range(3)]
            wit_t = [load(wit_d[fbase[f]:fbase[f] + FT_SIZES[f], :], (FT_SIZES[f], S), f"wit{f}", pool=fftp) for f in range(3)]

            # ---- phase D: FFT filter + residual + layernorms ----
            pr_sb, pi_sb = [], []
            for f in range(3):
                fs = FT_SIZES[f]
                fsl = slice(fbase[f], fbase[f] + fs)
                rtp = miscp.tile([128, S], dt, tag="m")
                for ht in range(NT):
                    mmr(rtp[0:fs, :], cret_t[ht][:, fsl], ctxt_sb[ht][:],
                                     start=(ht == 0), stop=(ht == NT - 1))
                rts = smallb.tile([128, S], dt, tag="rts")
                nc.vector.tensor_copy(rts[0:fs, :], rtp[0:fs, :])
                itp = miscp.tile([128, S], dt, tag="m")
                for ht in range(NT):
                    mmr(itp[0:fs, :], cimt_t[ht][:, fsl], ctxt_sb[ht][:],
                                     start=(ht == 0), stop=(ht == NT - 1))
                its = smallb.tile([128, S], dt, tag="its")
                nc.vector.tensor_copy(its[0:fs, :], itp[0:fs, :])
                t1 = smallb.tile([128, S], dt, tag="f1")
                t2 = smallb.tile([128, S], dt, tag="f2")
                nc.gpsimd.tensor_mul(t1[0:fs, :], rts[0:fs, :], wrt_t[f][:])
                nc.gpsimd.tensor_mul(t2[0:fs, :], its[0:fs, :], wit_t[f][:])
                pr = fftp.tile([fs, S], dt, tag=f"pr{f}", name=f"pr{f}")
                nc.vector.tensor_sub(pr[:].bitcast(R), t1[0:fs, :], t2[0:fs, :])
                pr_sb.append(pr)
                nc.gpsimd.tensor_mul(t1[0:fs, :], rts[0:fs, :], wit_t[f][:])
                nc.gpsimd.tensor_mul(t2[0:fs, :], its[0:fs, :], wrt_t[f][:])
                pi = fftp.tile([fs, S], dt, tag=f"pi{f}", name=f"pi{f}")
                nc.vector.tensor_add(pi[:].bitcast(R), t1[0:fs, :], t2[0:fs, :])
                pi_sb.append(pi)

            for st in range(NT):
                ssl = slice(st * 128, (st + 1) * 128)
                yp = miscp.tile([128, S], dt, tag="m")
                for f in range(3):
                    mmr(yp[:], pr_sb[f][:, ssl], irA_t[f][:], start=(f == 0), stop=False)
                    mmr(yp[:], pi_sb[f][:, ssl], irB_t[f][:], start=False, stop=False)
                for ht in range(NT):
                    mmr(yp[:, ht * 128:(ht + 1) * 128], ctxt_sb[ht][:, ssl], i128r[:],
                                     start=False, stop=(ht == NT - 1))

                def layer_norm(dst, src, wname, bname, tagn):
                    st6 = small.tile([128, 6], dt, tag="st6" + tagn)
                    nc.vector.bn_stats(st6[:], src)
                    mv = small.tile([128, 2], dt, tag="mv" + tagn)
                    nc.vector.bn_aggr(mv[:], st6[:])
                    lnv = small.tile([128, 1], dt, tag="lnv" + tagn)
                    nc.scalar.activation(lnv[:], mv[:, 1:2], AF.Ln,
                                         bias=constcol(1e-12)[:, 0:1], scale=1.0)
                    rs = small.tile([128, 1], dt, tag="rs" + tagn)
                    nc.scalar.activation(rs[:], lnv[:], AF.Exp, scale=-0.5)
                    nb = small.tile([128, 1], dt, tag="nb" + tagn)
                    nc.vector.scalar_tensor_tensor(
                        nb[:], mv[:, 0:1], -1.0, rs[:],
                        op0=mybir.AluOpType.mult, op1=mybir.AluOpType.mult)
                    nc.scalar.activation(dst, src, AF.Identity, bias=nb[:, 0:1], scale=rs[:, 0:1])
                    if flags["use_" + wname]:
                        nc.vector.tensor_mul(dst, dst, ln_bc[wname][:])
                    if flags["use_" + bname]:
                        nc.vector.tensor_add(dst, dst, ln_bc[bname][:])

                hid = work.tile([128, S], dt, tag="hid")
                layer_norm(hid[:], yp[:], "lnfw", "lnfb", "a")
                r2 = work.tile([128, S], dt, tag="r2")
                nc.vector.tensor_add(r2[:], hid[:], x_t[st][:])
                osb = work.tile([128, S], dt, tag="osb")
                layer_norm(osb[:], r2[:], "lnw", "lnb", "b")
                nc.sync.dma_start(out_d[ssl, :], osb[:])

    nsplit = _split_excess_waits(nc)
    if nsplit:
        print(f"[kernel] split {nsplit} excess sync waits onto NOPs")
    return nc

_CACHE = {}
LAST_EXEC_NS = None
LAST_RESULTS = None


def kernel(**inputs):
    inputs = {k: np.asarray(v) for k, v in inputs.items()}
    x_all = inputs["input_tensor"].astype(np.float32)
    mask = inputs["attention_mask"].astype(np.float32)
    cw = inputs["complex_weight"].astype(np.float32)

    flags = {
        "use_mask": bool(np.any(mask != 0)),
        "use_bq": bool(np.any(inputs["bq"] != 0)),
        "use_bk": bool(np.any(inputs["bk"] != 0)),
        "use_bv": bool(np.any(inputs["bv"] != 0)),
        "use_lnfw": not bool(np.all(inputs["ln_f_w"] == 1.0)),
        "use_lnfb": bool(np.any(inputs["ln_f_b"] != 0)),
        "use_lnw": not bool(np.all(inputs["ln_w"] == 1.0)),
        "use_lnb": bool(np.any(inputs["ln_b"] != 0)),
    }
    cvals = {
        "c": float(inputs["scalar"][0]) ** 2 / 2.0,
        "b_order": float(inputs["b_order"][0]),
        "b_dist": float(inputs["b_dist"][0]),
    }

    key = (tuple(sorted(flags.items())), tuple(sorted(cvals.items())))
    if key not in _CACHE:
        _CACHE[key] = _build_program(cvals, flags)
    nc = _CACHE[key]

    hc = _host_constants()
    wblk = np.zeros((H, 32), np.float32)
    hid_idx = np.arange(H)
    wblk[hid_idx, hid_idx // D] = inputs["W_order"][:D, 0][hid_idx % D]
    wblk[hid_idx, 8 + hid_idx // D] = inputs["W_order"][D:, 0][hid_idx % D]
    wblk[hid_idx, 16 + hid_idx // D] = inputs["W_dist"][:D, 0][hid_idx % D]
    wblk[hid_idx, 24 + hid_idx // D] = inputs["W_dist"][D:, 0][hid_idx % D]

    shared = {
        "wq": inputs["Wq"].astype(np.float32),
        "wk": inputs["Wk"].astype(np.float32),
        "wv": inputs["Wv"].astype(np.float32),
        "wblk": wblk,
        "g2": hc["g2"], "ss": hc["ss"],
        "gh": hc["g"].astype(np.float16),
        "cret": hc["cret"], "cimt": hc["cimt"],
        "irA": hc["irA"], "irB": hc["irB"],
        "wrt": np.ascontiguousarray(cw[0, :, :, 0].T),
        "wit": np.ascontiguousarray(cw[0, :, :, 1].T),
        "onesel": hc["onesel"],
        "halfsel": hc["halfsel"],
    }
    if flags["use_bq"]:
        shared["bq"] = inputs["bq"].astype(np.float32)
    if flags["use_bk"]:
        shared["bk"] = inputs["bk"].astype(np.float32)
    if flags["use_bv"]:
        shared["bv"] = inputs["bv"].astype(np.float32)
    for nm, src in (("lnfw", "ln_f_w"), ("lnfb", "ln_f_b"), ("lnw", "ln_w"), ("lnb", "ln_b")):
        if flags["use_" + nm]:
            shared[nm] = inputs[src].astype(np.float32)

    in_maps = []
    for b in range(B):
        m = dict(shared)
        m["x"] = np.ascontiguousarray(x_all[b])
        if flags["use_mask"]:
            m["m8"] = np.ascontiguousarray(8.0 * mask[b, 0, 0, :])
        in_maps.append(m)

    import os
    trace = os.environ.get("KERNEL_TRACE", "") == "1"
    res = run_bass_kernel_spmd(nc, in_maps, core_ids=list(range(B)), trace=trace)
    global LAST_EXEC_NS, LAST_RESULTS
    LAST_RESULTS = res
    if res.exec_time_ns is not None:
        LAST_EXEC_NS = res.exec_time_ns
    out = np.stack([res.results[b]["out"] for b in range(B)]).astype(np.float32)
    return out


if __name__ == "__main__":
    rng = np.random.default_rng(0)
    print("kernel module ok")

